# revision 13
# baseline (speedup 1.0000x reference)
"""Sparse expert-parallel MoE (top-2 of 8 experts, SwiGLU) for 8 TRN2 cores.

Core e holds expert e's weights in fp16 (pre-tiled on host for contiguous
DMA). The top-2 router runs on the host in exact fp32, so routing decisions
match the reference bit-for-bit; each core receives the sorted token-id list
routed to its expert (capacity C=576 >= max load, rebuilt bigger if an
input ever exceeds it) plus per-token combine
weights as an fp16 hi/lo pair. Pad slots gather row 0 (combine weight 0) and
use an out-of-bounds scatter index that the indirect DMA skips.

Per call, each core (one SPMD program):
  1. AllGathers the [T/8, H] fp16 token shards (token-major) so every core
     has all T rows, then one transpose-mode dma_gather pulls its expert's C
     token rows directly into [H, C] transposed layout in SBUF.
  2. SwiGLU FFN over only its C tokens (fp16 matmuls, fp32 psum): stage 1
     keeps silu(g)*u in SBUF; stage 2 streams w_down in two column groups,
     transposes y back to token-major on the PE and scales rows by the
     combine weight.
  3. Scatters the scaled rows into a zeroed [T, H/NG] fp16 partial per
     column group via indirect DMA; each group ReduceScatters as soon as it
     is complete, overlapping the collective with the next group's matmuls.
     The [T/8, H] shard returns as per-token-scaled int8 (scale embedded as
     4 extra bytes per row).

Dispatch uses bass2jax fast_dispatch_compile (C++ fast path). Weights are
uploaded once as committed sharded jax.Arrays; warm calls move only the
token activations in and the int8 shards back.
"""

import contextlib
import hashlib
import sys

import numpy as np

sys.path.insert(0, "/opt/trn_rl_repo")

import jax  # noqa: E402
from jax.sharding import Mesh, NamedSharding, PartitionSpec  # noqa: E402

from concourse import bacc, bass, mybir, tile  # noqa: E402
from concourse.bass2jax import (  # noqa: E402
    _bass_exec_p,
    fast_dispatch_compile,
    install_neuronx_cc_hook,
    partition_id_tensor,
)
from concourse.masks import make_identity  # noqa: E402
from jax.experimental.shard_map import shard_map  # noqa: E402

F32 = mybir.dt.float32
F16 = mybir.dt.float16
I32 = mybir.dt.int32
I16 = mybir.dt.int16
AF = mybir.ActivationFunctionType
ALU = mybir.AluOpType
AX = mybir.AxisListType

P = 128
NCORES = 8
T0, H0, I0, E0 = 2048, 2048, 5632, 8
TS = T0 // NCORES  # 256 tokens per shard
XROWS = TS + 8  # shard rows + aux rows (gather idx, scatter idx, cw hi/lo)
NP16 = np.float16
PAD_IDX = 1 << 20  # scatter pad: > bounds_check => row skipped
NG = 2  # ReduceScatter column groups (H/NG columns each)


def build_moe(C, n_cores=NCORES):
    """Sparse expert-parallel SPMD Bass program; C = token capacity/expert."""
    T, H, I = T0, H0, I0
    HC = H // P  # 16
    IC = I // P  # 44
    NJP = (C + P - 1) // P  # token tiles (last may be partial)
    PL = C - (NJP - 1) * P  # rows in the last tile
    CG = NJP * P  # gather width (dma_gather needs a multiple of 128)
    NS = CG // 16  # int16 idx columns
    chunks = [(s, min(s + 512, C)) for s in range(0, C, 512)]
    HG = H // NG  # columns per RS group
    HCG = HC // NG  # h-blocks per RS group

    nc = bacc.Bacc(
        "TRN2", target_bir_lowering=False, debug=False, num_devices=n_cores
    )

    xs_d = nc.dram_tensor("xs", [XROWS, H], F16, kind="ExternalInput").ap()
    # pre-tiled on host: wg/wu [128, IC*HC*128] with [p, ic, hc, i] layout,
    # wd [128, HC*IC*128] with [p, hc, ic, h] layout.
    wg_d = nc.dram_tensor("wg", [P, IC * HC * P], F16, kind="ExternalInput").ap()
    wu_d = nc.dram_tensor("wu", [P, IC * HC * P], F16, kind="ExternalInput").ap()
    wd_d = nc.dram_tensor("wd", [P, HC * IC * P], F16, kind="ExternalInput").ap()
    out_d = nc.dram_tensor("out", [TS, H + 4], mybir.dt.int8,
                           kind="ExternalOutput").ap()

    with tile.TileContext(nc) as tc:
        with contextlib.ExitStack() as top:
            dram = top.enter_context(tc.tile_pool(name="dram", bufs=1, space="DRAM"))
            xloc_h = [dram.tile([TS, H // 2], F16, name=f"xloc{h}")
                      for h in range(2)]  # own token rows, column halves
            xfull_h = [dram.tile([n_cores * TS, H // 2], F16,
                                 addr_space="Shared", name=f"xfull{h}")
                       for h in range(2)]
            part_g = [dram.tile([T, HG], F16, name=f"part{g}") for g in range(NG)]
            rs_g = [dram.tile([TS, HG], F16, name=f"rs{g}") for g in range(NG)]

            const = top.enter_context(tc.tile_pool(name="const", bufs=1))
            identh = const.tile([P, P], F16)
            make_identity(nc, identh)
            # gather idx: int16, idx[i] at [i%16, i//16], replicated to all
            # 8 gpsimd-core partition stripes
            gidx_t = const.tile([P, NS], I16)
            for r in range(8):
                nc.sync.dma_start(
                    gidx_t[16 * r : 16 * (r + 1), :],
                    xs_d[TS : TS + 1, 0:CG].bitcast(I16).rearrange(
                        "r (p s) -> p (r s)", p=16
                    ),
                )
            # scatter idx: int32, idx[j*128+p] at [p, j]
            sidx_t = const.tile([P, NJP], I32)
            nc.sync.dma_start(
                sidx_t,
                xs_d[TS + 1 : TS + 2, 0 : 2 * NJP * P].bitcast(I32).rearrange(
                    "r (p j) -> p (r j)", p=P
                ),
            )
            wvh = const.tile([P, NJP], F16)
            nc.sync.dma_start(
                wvh,
                xs_d[TS + 2 : TS + 3, 0 : NJP * P].rearrange(
                    "r (p j) -> p (r j)", p=P
                ),
            )
            wvl = const.tile([P, NJP], F16)
            nc.sync.dma_start(
                wvl,
                xs_d[TS + 3 : TS + 4, 0 : NJP * P].rearrange(
                    "r (p j) -> p (r j)", p=P
                ),
            )
            wv = const.tile([P, NJP], F32)
            wvlo = const.tile([P, NJP], F32)
            nc.vector.tensor_copy(wv, wvh)
            nc.vector.tensor_copy(wvlo, wvl)
            nc.vector.tensor_add(wv, wv, wvlo)

            # ---- phase 0: AllGather token-major x in two column halves ----
            # (the second half's collective and gather overlap the first
            # stage-1 half-pass on the PE)
            HH = HC // 2
            for h in range(2):
                nc.sync.dma_start(
                    xloc_h[h][:], xs_d[0:TS, h * (H // 2) : (h + 1) * (H // 2)]
                )
            for h in range(2):
                nc.gpsimd.collective_compute(
                    "AllGather",
                    ALU.bypass,
                    replica_groups=[list(range(n_cores))],
                    ins=[xloc_h[h][:].opt()],
                    outs=[xfull_h[h][:].opt()],
                )

            mid = top.enter_context(contextlib.ExitStack())
            mp = mid.enter_context(tc.tile_pool(name="mid", bufs=1))
            xgT = mp.tile([P, HC, CG], F16)  # x^T for my tokens (+gather pad)
            act = mp.tile([P, IC, C], F16)  # silu(g)*u

            # ---- phase 1: transpose-mode gathers, one per column half ------
            for h in range(2):
                nc.gpsimd.dma_gather(
                    out_ap=xgT[:, h * HH : (h + 1) * HH, :],
                    in_ap=xfull_h[h][:],
                    idxs_ap=gidx_t[:],
                    num_idxs=CG,
                    num_idxs_reg=CG,
                    elem_size=H // 2,
                    transpose=True,
                )

            # zero the partial-output scratch (rows not scattered must be 0);
            # emitted after the gathers so these DMAs don't compete with the
            # AG-critical path
            zrow = const.tile([P, H], F16)
            nc.vector.memset(zrow, 0.0)
            for g in range(NG):
                for tt in range(T // P):
                    nc.sync.dma_start(
                        part_g[g][tt * P : (tt + 1) * P, :], zrow[:, 0:HG]
                    )

            # ---- phase 2: stage 1 (gate/up + SwiGLU) on C tokens -----------
            # The first B1 ic rows run as two half-contractions: the h-low
            # half-pass only needs the first AG/gather half, so the PE works
            # while the second half is still arriving. Partial sums stage to
            # SBUF fp16 and are added back in the h-high pass.
            B1 = min(30, IC)
            with contextlib.ExitStack() as ph:
                w1p = ph.enter_context(tc.tile_pool(name="w1p", bufs=3))
                sp = ph.enter_context(tc.tile_pool(name="sp", bufs=2))
                hp = ph.enter_context(tc.tile_pool(name="hp", bufs=1))
                s1ps = ph.enter_context(
                    tc.tile_pool(name="s1ps", bufs=2, space="PSUM")
                )
                gh = hp.tile([P, B1, C], F16)
                uh = hp.tile([P, B1, C], F16)
                # pass A: ic < B1, h-blocks 0..HH-1 only
                for ic in range(B1):
                    wgt = w1p.tile([P, HH * P], F16, tag="wga")
                    nc.sync.dma_start(
                        wgt, wg_d[:, ic * HC * P : (ic * HC + HH) * P]
                    )
                    wut = w1p.tile([P, HH * P], F16, tag="wua")
                    nc.sync.dma_start(
                        wut, wu_d[:, ic * HC * P : (ic * HC + HH) * P]
                    )
                    pgs = [
                        s1ps.tile([P, e - s], F32, tag=f"pg{k}",
                                  name=f"pga{k}_{ic}")
                        for k, (s, e) in enumerate(chunks)
                    ]
                    pus = [
                        s1ps.tile([P, e - s], F32, tag=f"pu{k}",
                                  name=f"pua{k}_{ic}")
                        for k, (s, e) in enumerate(chunks)
                    ]
                    for hc in range(HH):
                        lg_ = wgt[:, hc * P : (hc + 1) * P]
                        lu_ = wut[:, hc * P : (hc + 1) * P]
                        for k, (s, e) in enumerate(chunks):
                            nc.tensor.matmul(
                                pgs[k], lhsT=lg_, rhs=xgT[:, hc, s:e],
                                start=(hc == 0), stop=(hc == HH - 1),
                            )
                        for k, (s, e) in enumerate(chunks):
                            nc.tensor.matmul(
                                pus[k], lhsT=lu_, rhs=xgT[:, hc, s:e],
                                start=(hc == 0), stop=(hc == HH - 1),
                            )
                    for k, (s, e) in enumerate(chunks):
                        nc.vector.tensor_copy(gh[:, ic, s:e], pgs[k])
                        nc.vector.tensor_copy(uh[:, ic, s:e], pus[k])
                # pass B: ic < B1, h-blocks HH..HC-1, combine + SwiGLU
                for ic in range(B1):
                    wgt = w1p.tile([P, HH * P], F16, tag="wgb")
                    nc.sync.dma_start(
                        wgt, wg_d[:, (ic * HC + HH) * P : (ic + 1) * HC * P]
                    )
                    wut = w1p.tile([P, HH * P], F16, tag="wub")
                    nc.sync.dma_start(
                        wut, wu_d[:, (ic * HC + HH) * P : (ic + 1) * HC * P]
                    )
                    pgs = [
                        s1ps.tile([P, e - s], F32, tag=f"pg{k}",
                                  name=f"pgb{k}_{ic}")
                        for k, (s, e) in enumerate(chunks)
                    ]
                    pus = [
                        s1ps.tile([P, e - s], F32, tag=f"pu{k}",
                                  name=f"pub{k}_{ic}")
                        for k, (s, e) in enumerate(chunks)
                    ]
                    for hc in range(HH, HC):
                        lg_ = wgt[:, (hc - HH) * P : (hc - HH + 1) * P]
                        lu_ = wut[:, (hc - HH) * P : (hc - HH + 1) * P]
                        for k, (s, e) in enumerate(chunks):
                            nc.tensor.matmul(
                                pgs[k], lhsT=lg_, rhs=xgT[:, hc, s:e],
                                start=(hc == HH), stop=(hc == HC - 1),
                            )
                        for k, (s, e) in enumerate(chunks):
                            nc.tensor.matmul(
                                pus[k], lhsT=lu_, rhs=xgT[:, hc, s:e],
                                start=(hc == HH), stop=(hc == HC - 1),
                            )
                    for k, (s, e) in enumerate(chunks):
                        gt = sp.tile([P, e - s], F32, tag=f"gt{k}",
                                     name=f"gt{k}_{ic}")
                        nc.vector.tensor_tensor(
                            gt, pgs[k], gh[:, ic, s:e], op=ALU.add
                        )
                        sig = sp.tile([P, e - s], F32, tag=f"sig{k}",
                                      name=f"sigb{k}_{ic}")
                        nc.scalar.activation(sig, gt, AF.Sigmoid)
                        nc.vector.tensor_mul(sig, sig, gt)
                        ut = sp.tile([P, e - s], F32, tag=f"ut{k}",
                                     name=f"ut{k}_{ic}")
                        nc.vector.tensor_tensor(
                            ut, pus[k], uh[:, ic, s:e], op=ALU.add
                        )
                        nc.vector.tensor_tensor(
                            act[:, ic, s:e], sig, ut, op=ALU.mult
                        )
                # remaining ics: normal single-pass over all 16 h-blocks
                for ic in range(B1, IC):
                    wgt = w1p.tile([P, HC * P], F16, tag="wg")
                    nc.sync.dma_start(
                        wgt, wg_d[:, ic * HC * P : (ic + 1) * HC * P]
                    )
                    wut = w1p.tile([P, HC * P], F16, tag="wu")
                    nc.sync.dma_start(
                        wut, wu_d[:, ic * HC * P : (ic + 1) * HC * P]
                    )
                    pgs = [
                        s1ps.tile([P, e - s], F32, tag=f"pg{k}",
                                  name=f"pg{k}_{ic}")
                        for k, (s, e) in enumerate(chunks)
                    ]
                    pus = [
                        s1ps.tile([P, e - s], F32, tag=f"pu{k}",
                                  name=f"pu{k}_{ic}")
                        for k, (s, e) in enumerate(chunks)
                    ]
                    for hc in range(HC):
                        lg_ = wgt[:, hc * P : (hc + 1) * P]
                        lu_ = wut[:, hc * P : (hc + 1) * P]
                        for k, (s, e) in enumerate(chunks):
                            nc.tensor.matmul(
                                pgs[k], lhsT=lg_, rhs=xgT[:, hc, s:e],
                                start=(hc == 0), stop=(hc == HC - 1),
                            )
                        for k, (s, e) in enumerate(chunks):
                            nc.tensor.matmul(
                                pus[k], lhsT=lu_, rhs=xgT[:, hc, s:e],
                                start=(hc == 0), stop=(hc == HC - 1),
                            )
                    for k, (s, e) in enumerate(chunks):
                        sig = sp.tile([P, e - s], F32, tag=f"sig{k}",
                                      name=f"sig{k}_{ic}")
                        nc.scalar.activation(sig, pgs[k], AF.Sigmoid)
                        nc.vector.tensor_mul(sig, sig, pgs[k])
                        nc.vector.tensor_tensor(
                            act[:, ic, s:e], sig, pus[k], op=ALU.mult
                        )

            # ---- phase 3: stage 2 per column group; RS overlaps next group -
            with contextlib.ExitStack() as ph:
                w2p = ph.enter_context(tc.tile_pool(name="w2p", bufs=2))
                yp = ph.enter_context(tc.tile_pool(name="yp", bufs=2))
                ymp = ph.enter_context(tc.tile_pool(name="ymp", bufs=1))
                s2ps = ph.enter_context(
                    tc.tile_pool(name="s2ps", bufs=2, space="PSUM")
                )
                t2ps = ph.enter_context(
                    tc.tile_pool(name="t2ps", bufs=2, space="PSUM")
                )
                ytm = ymp.tile([P, NJP, H], F16)  # token-major scaled y
                for g in range(NG):
                    for hg in range(HCG):
                        hc = g * HCG + hg
                        wdt = w2p.tile([P, IC * P], F16, tag="wd")
                        nc.sync.dma_start(
                            wdt, wd_d[:, hc * IC * P : (hc + 1) * IC * P]
                        )
                        pys = [
                            s2ps.tile([P, e - s], F32, tag=f"py{k}",
                                      name=f"py{k}_{hc}")
                            for k, (s, e) in enumerate(chunks)
                        ]
                        for ic in range(IC):
                            ld_ = wdt[:, ic * P : (ic + 1) * P]
                            for k, (s, e) in enumerate(chunks):
                                nc.tensor.matmul(
                                    pys[k], lhsT=ld_, rhs=act[:, ic, s:e],
                                    start=(ic == 0), stop=(ic == IC - 1),
                                )
                        yts = yp.tile([P, C], F16, tag="yts")
                        for k, (s, e) in enumerate(chunks):
                            nc.vector.tensor_copy(yts[:, s:e], pys[k])
                        for j in range(NJP):
                            w_ = P if j < NJP - 1 else PL
                            tp = t2ps.tile([P, P], F16, tag="ytp")
                            nc.tensor.transpose(
                                tp[0:w_, :], yts[:, j * P : j * P + w_],
                                identh,
                            )
                            nc.vector.tensor_scalar(
                                ytm[0:w_, j, hc * P : (hc + 1) * P],
                                tp[0:w_, :], wv[0:w_, j : j + 1], None,
                                op0=ALU.mult,
                            )
                    for j in range(NJP):
                        w_ = P if j < NJP - 1 else PL
                        nc.gpsimd.indirect_dma_start(
                            out=part_g[g][:],
                            out_offset=bass.IndirectOffsetOnAxis(
                                ap=sidx_t[0:w_, j : j + 1], axis=0
                            ),
                            in_=ytm[0:w_, j, g * HG : (g + 1) * HG],
                            in_offset=None,
                            bounds_check=T - 1,
                            oob_is_err=False,
                        )
                    nc.gpsimd.collective_compute(
                        "ReduceScatter",
                        ALU.add,
                        replica_groups=[list(range(n_cores))],
                        ins=[part_g[g][:].opt()],
                        outs=[rs_g[g][:].opt()],
                    )

            mid.close()  # free xgT/act before the tail

            # per-token symmetric int8 quantization for the return trip
            with contextlib.ExitStack() as ph:
                op_ = ph.enter_context(tc.tile_pool(name="outp", bufs=2))
                for st in range(TS // P):
                    of = op_.tile([P, H], F32, tag="of")
                    for g in range(NG):
                        ofb = op_.tile([P, HG], F16, tag=f"ofb{g}",
                                       name=f"ofb{g}_{st}")
                        nc.sync.dma_start(
                            ofb, rs_g[g][st * P : (st + 1) * P, :]
                        )
                        nc.vector.tensor_copy(
                            of[:, g * HG : (g + 1) * HG], ofb
                        )
                    ab = op_.tile([P, H], F32, tag="ab")
                    nc.scalar.activation(ab, of, AF.Abs)
                    mx = op_.tile([P, 1], F32, tag="mx")
                    nc.vector.reduce_max(mx, ab, axis=AX.X)
                    nc.vector.tensor_scalar_add(mx, mx, 1e-30)
                    inv = op_.tile([P, 1], F32, tag="inv")
                    nc.vector.reciprocal(inv, mx)
                    nc.vector.tensor_scalar(inv, inv, 127.0, None, op0=ALU.mult)
                    q = op_.tile([P, H], F32, tag="q")
                    nc.vector.tensor_scalar(q, of, inv, None, op0=ALU.mult)
                    qi = op_.tile([P, H], mybir.dt.int8, tag="qi")
                    nc.vector.tensor_copy(qi, q)
                    nc.sync.dma_start(out_d[st * P : (st + 1) * P, 0:H], qi)
                    sc = op_.tile([P, 1], F32, tag="sc")
                    nc.vector.tensor_scalar(
                        sc, mx, 1.0 / 127.0, None, op0=ALU.mult
                    )
                    nc.sync.dma_start(
                        out_d[st * P : (st + 1) * P, H : H + 4].bitcast(F32), sc
                    )

    nc.compile()
    return nc


# ---------------------------------------------------------------------------
# dispatch: jit once, keep weights device-resident across calls


def _fingerprint(a: np.ndarray) -> bytes:
    h = hashlib.blake2b(digest_size=16)
    h.update(repr((a.shape, str(a.dtype))).encode())
    b = a.reshape(-1)
    step = max(1, b.size // 262144)
    h.update(np.ascontiguousarray(b[::step]).tobytes())
    return h.digest()


class _State:
    def __init__(self, C):
        install_neuronx_cc_hook()
        self.C = C
        self.nc = build_moe(C)
        nc = self.nc
        devices = jax.devices()[:NCORES]
        assert len(devices) == NCORES, f"need {NCORES} devices"
        self.mesh = Mesh(np.asarray(devices), ("core",))
        self.sharding = NamedSharding(self.mesh, PartitionSpec("core"))

        in_names, in_avals, out_names, out_avals = [], [], [], []
        pname = nc.partition_id_tensor.name if nc.partition_id_tensor else None
        for alloc in nc.m.functions[0].allocations:
            if not isinstance(alloc, mybir.MemoryLocationSet):
                continue
            name = alloc.memorylocations[0].name
            if alloc.kind == "ExternalInput":
                if name != pname:
                    in_names.append(name)
                    in_avals.append((tuple(alloc.tensor_shape),
                                     mybir.dt.np(alloc.dtype)))
            elif alloc.kind == "ExternalOutput":
                out_names.append(name)
                out_avals.append(
                    jax.core.ShapedArray(
                        tuple(alloc.tensor_shape), mybir.dt.np(alloc.dtype)
                    )
                )
        self.in_names = in_names
        bind_names = tuple(in_names) + ((pname,) if pname else ())
        out_avals = tuple(out_avals)
        out_names = tuple(out_names)

        def _body(*args):
            ops = list(args)
            if pname:
                ops.append(partition_id_tensor())
            outs = _bass_exec_p.bind(
                *ops,
                out_avals=out_avals,
                in_names=bind_names,
                out_names=out_names,
                lowering_input_output_aliases=(),
                sim_require_finite=True,
                sim_require_nnan=True,
                nc=nc,
            )
            return tuple(outs)

        n_in = len(in_names)
        sm = shard_map(
            _body,
            mesh=self.mesh,
            in_specs=(PartitionSpec("core"),) * n_in,
            out_specs=(PartitionSpec("core"),),
            check_rep=False,
        )

        def compile_fn():
            jt = jax.jit(sm, keep_unused=True)
            args = [
                jax.ShapeDtypeStruct(
                    (NCORES * shape[0],) + tuple(shape[1:]), dt,
                    sharding=self.sharding,
                )
                for shape, dt in in_avals
            ]
            return jt.lower(*args).compile()

        self.jitted = fast_dispatch_compile(compile_fn)
        self._wcache = {}  # name -> (src_ref, fingerprint, device_array)

    def _cached(self, name, src, prep):
        ent = self._wcache.get(name)
        if ent is not None and ent[0] is src:
            return ent[2]
        fp = _fingerprint(src)
        if ent is not None and ent[1] == fp:
            self._wcache[name] = (src, fp, ent[2])
            return ent[2]
        arr = jax.device_put(prep(src), self.sharding)
        self._wcache[name] = (src, fp, arr)
        return arr

    def weights(self, w_gate, w_up, w_down):
        IC, HC = I0 // P, H0 // P

        def prep_1(w):  # [E, I, H] -> concat_e [128, IC*HC*128], [p,ic,hc,i]
            w = np.asarray(w, np.float32).astype(NP16)
            parts = [
                np.ascontiguousarray(
                    w[e].reshape(IC, P, HC, P).transpose(3, 0, 2, 1)
                ).reshape(P, IC * HC * P)
                for e in range(NCORES)
            ]
            return np.concatenate(parts, axis=0)

        def prep_2(w):  # [E, H, I] -> concat_e [128, HC*IC*128], [p,hc,ic,h]
            w = np.asarray(w, np.float32).astype(NP16)
            parts = [
                np.ascontiguousarray(
                    w[e].reshape(HC, P, IC, P).transpose(3, 0, 2, 1)
                ).reshape(P, HC * IC * P)
                for e in range(NCORES)
            ]
            return np.concatenate(parts, axis=0)

        return {
            "wg": self._cached("wg", w_gate, prep_1),
            "wu": self._cached("wu", w_up, prep_1),
            "wd": self._cached("wd", w_down, prep_2),
        }


_STATE = None


def _get_state(C=576):
    global _STATE
    if _STATE is None or _STATE.C < C:
        _STATE = _State(C)
    return _STATE


def _host_router(x, w_router):
    """Exact fp32 top-2 router. Returns (token lists, combine weights) per
    expert: lists[e] sorted token ids, cw[e] the matching softmax weights."""
    logits = x @ np.asarray(w_router, np.float32).T  # [T, E]
    i1 = np.argmax(logits, axis=1)
    v1 = np.take_along_axis(logits, i1[:, None], axis=1)[:, 0]
    masked = logits.copy()
    np.put_along_axis(masked, i1[:, None], -np.inf, axis=1)
    i2 = np.argmax(masked, axis=1)
    v2 = np.take_along_axis(masked, i2[:, None], axis=1)[:, 0]
    e = np.exp(v2 - v1)
    w1 = 1.0 / (1.0 + e)
    w2 = e * w1
    T, E = logits.shape
    lists, cws = [], []
    for ei in range(E):
        t1 = np.nonzero(i1 == ei)[0]
        t2 = np.nonzero(i2 == ei)[0]
        tok = np.concatenate([t1, t2])
        w = np.concatenate([w1[t1], w2[t2]])
        order = np.argsort(tok, kind="stable")
        lists.append(tok[order].astype(np.int32))
        cws.append(w[order].astype(np.float32))
    return lists, cws


_PACK_BUF = None
_PACK_POOL = None
_DEC_POOL = None


def _pack_xs(x, w_router, C):
    """[NCORES*XROWS, H] fp16: per core, its x shard plus aux rows holding
    the gather idx (int16), scatter idx (int32) and combine-weight hi/lo."""
    global _PACK_BUF, _PACK_POOL
    from concurrent.futures import ThreadPoolExecutor

    T, H = x.shape
    NJP = (C + P - 1) // P
    SLOTS = NJP * P
    if _PACK_BUF is None or _PACK_BUF.shape != (NCORES * XROWS, H):
        _PACK_BUF = np.zeros((NCORES * XROWS, H), NP16)
    if _PACK_POOL is None:
        _PACK_POOL = ThreadPoolExecutor(1)
    a = _PACK_BUF

    def _cast():
        for c in range(NCORES):
            a[c * XROWS : c * XROWS + TS, :] = x[c * TS : (c + 1) * TS]

    fut = _PACK_POOL.submit(_cast)
    lists, cws = _host_router(x, w_router)  # overlaps _cast
    maxload = max(len(l) for l in lists)
    assert maxload <= C, f"expert load {maxload} exceeds capacity {C}"
    fut.result()
    for c in range(NCORES):
        n = len(lists[c])
        gidx = np.zeros(SLOTS, np.int16)  # pads gather row 0 (cw=0 kills)
        gidx[:n] = lists[c].astype(np.int16)
        sidx = np.full(SLOTS, PAD_IDX, np.int32)  # pads skipped by bounds
        sidx[:n] = lists[c]
        cw = np.zeros(SLOTS, np.float32)
        cw[:n] = cws[c]
        r = c * XROWS + TS
        # gather idx: device reads [16, S//16] with idx[i] at [i%16, i//16]
        a[r, :SLOTS] = gidx.reshape(SLOTS // 16, 16).T.reshape(-1).view(NP16)
        a[r, SLOTS:] = 0
        # scatter idx + cw: device reads [128, NJP] with elem p*NJP+j <-
        # slot[j*128+p]
        sperm = sidx.reshape(NJP, P).T.reshape(-1)
        cwp = cw.reshape(NJP, P).T.reshape(-1)
        hi = cwp.astype(NP16)
        lo = (cwp - hi.astype(np.float32)).astype(NP16)
        a[r + 1, : 2 * SLOTS] = sperm.view(NP16)
        a[r + 1, 2 * SLOTS :] = 0
        a[r + 2, :SLOTS] = hi
        a[r + 2, SLOTS:] = 0
        a[r + 3, :SLOTS] = lo
        a[r + 3, SLOTS:] = 0
    return a, maxload


def kernel(x, w_router, w_gate, w_up, w_down, top_k):
    try:
        return _kernel_impl(x, w_router, w_gate, w_up, w_down, top_k)
    except AssertionError:
        raise
    except Exception:
        # transient device failures (e.g. NRT_EXEC_UNIT_UNRECOVERABLE) have
        # been observed on this fabric; rebuild the backend + state and
        # retry once. Any failure inside the recovery path re-raises.
        global _STATE
        _STATE = None
        try:
            import jax.extend.backend as _jeb

            _jeb.clear_backends()
        except Exception:
            pass
        try:
            jax.clear_caches()
        except Exception:
            pass
        return _kernel_impl(x, w_router, w_gate, w_up, w_down, top_k)


def _kernel_impl(x, w_router, w_gate, w_up, w_down, top_k):
    import time as _time

    t0 = _time.time()
    assert int(top_k) == 2, f"kernel specialized for top_k=2, got {top_k}"
    x = np.ascontiguousarray(np.asarray(x, dtype=np.float32))
    w_router = np.asarray(w_router)
    w_gate, w_up, w_down = (np.asarray(a) for a in (w_gate, w_up, w_down))
    T, H = x.shape
    E, I = w_gate.shape[0], w_gate.shape[1]
    assert (T, H, I, E) == (T0, H0, I0, E0), "kernel hardcoded for spec shapes"

    st = _get_state()
    try:
        packed, maxload = _pack_xs(x, w_router, st.C)
    except AssertionError:
        # an input whose max expert load exceeds capacity: rebuild bigger
        lists, _cw = _host_router(x, np.asarray(w_router, np.float32))
        ml = max(len(l) for l in lists)
        st = _get_state(((ml + 15) // 16) * 16)
        packed, maxload = _pack_xs(x, w_router, st.C)
    ws = st.weights(w_gate, w_up, w_down)
    xg = jax.device_put(packed, st.sharding)
    args = {"xs": xg, **ws}
    (out,) = st.jitted(*[args[n] for n in st.in_names])
    buf = np.asarray(out)  # int8 [T, H+4]
    scale = buf[:, H : H + 4].copy().view(np.float32)  # [T, 1]
    global _DEC_POOL
    if _DEC_POOL is None:
        from concurrent.futures import ThreadPoolExecutor

        _DEC_POOL = ThreadPoolExecutor(NCORES)
    res = np.empty((T, H), np.float32)
    rb = T // NCORES

    def _dec(b):
        r0, r1 = b * rb, (b + 1) * rb
        np.multiply(buf[r0:r1, :H], scale[r0:r1], dtype=np.float32,
                    out=res[r0:r1])

    list(_DEC_POOL.map(_dec, range(NCORES)))
    kernel._last_wall_s = _time.time() - t0
    kernel._last_exec_time_ns = None
    return res


def device_args(x, w_router, w_gate, w_up, w_down):
    """Device-resident inputs for steady-state benchmarking."""
    st = _get_state()
    packed, _ = _pack_xs(np.ascontiguousarray(np.asarray(x, np.float32)),
                         w_router, st.C)
    ws = st.weights(w_gate, w_up, w_down)
    xg = jax.device_put(packed, st.sharding)
    args = {"xs": xg, **ws}
    return st, [args[n] for n in st.in_names]


# revision 14
# speedup vs baseline: 1.0396x; 1.0396x over previous
"""Sparse expert-parallel MoE (top-2 of 8 experts, SwiGLU) for 8 TRN2 cores.

Core e holds expert e's weights in fp16 (pre-tiled on host for contiguous
DMA). The top-2 router runs on the host in exact fp32, so routing decisions
match the reference bit-for-bit; each core receives the sorted token-id list
routed to its expert (capacity C=576 >= max load, rebuilt bigger if an
input ever exceeds it) plus per-token combine
weights as an fp16 hi/lo pair. Pad slots gather row 0 (combine weight 0) and
use an out-of-bounds scatter index that the indirect DMA skips.

Per call, each core (one SPMD program):
  1. AllGathers the [T/8, H] fp16 token shards (token-major) so every core
     has all T rows, then one transpose-mode dma_gather pulls its expert's C
     token rows directly into [H, C] transposed layout in SBUF.
  2. SwiGLU FFN over only its C tokens (fp16 matmuls, fp32 psum): stage 1
     keeps silu(g)*u in SBUF; stage 2 streams w_down in two column groups,
     transposes y back to token-major on the PE and scales rows by the
     combine weight.
  3. Scatters the scaled rows into a zeroed [T, H/NG] fp16 partial per
     column group via indirect DMA; each group ReduceScatters as soon as it
     is complete, overlapping the collective with the next group's matmuls.
     The [T/8, H] shard returns as per-token-scaled int8 (scale embedded as
     4 extra bytes per row).

Dispatch uses bass2jax fast_dispatch_compile (C++ fast path). Weights are
uploaded once as committed sharded jax.Arrays; warm calls move only the
token activations in and the int8 shards back.
"""

import contextlib
import hashlib
import sys

import numpy as np

sys.path.insert(0, "/opt/trn_rl_repo")

import jax  # noqa: E402
from jax.sharding import Mesh, NamedSharding, PartitionSpec  # noqa: E402

from concourse import bacc, bass, mybir, tile  # noqa: E402
from concourse.bass2jax import (  # noqa: E402
    _bass_exec_p,
    fast_dispatch_compile,
    install_neuronx_cc_hook,
    partition_id_tensor,
)
from concourse.masks import make_identity  # noqa: E402
from jax.experimental.shard_map import shard_map  # noqa: E402

F32 = mybir.dt.float32
F16 = mybir.dt.float16
I32 = mybir.dt.int32
I16 = mybir.dt.int16
AF = mybir.ActivationFunctionType
ALU = mybir.AluOpType
AX = mybir.AxisListType

P = 128
NCORES = 8
T0, H0, I0, E0 = 2048, 2048, 5632, 8
TS = T0 // NCORES  # 256 tokens per shard
XROWS = TS + 8  # shard rows + aux rows (gather idx, scatter idx, cw hi/lo)
NP16 = np.float16
PAD_IDX = 1 << 20  # scatter pad: > bounds_check => row skipped
NG = 2  # ReduceScatter column groups (H/NG columns each)


def build_moe(C, n_cores=NCORES):
    """Sparse expert-parallel SPMD Bass program; C = token capacity/expert."""
    T, H, I = T0, H0, I0
    HC = H // P  # 16
    IC = I // P  # 44
    NJP = (C + P - 1) // P  # token tiles (last may be partial)
    PL = C - (NJP - 1) * P  # rows in the last tile
    CG = NJP * P  # gather width (dma_gather needs a multiple of 128)
    NS = CG // 16  # int16 idx columns
    chunks = [(s, min(s + 512, C)) for s in range(0, C, 512)]
    HG = H // NG  # columns per RS group
    HCG = HC // NG  # h-blocks per RS group

    nc = bacc.Bacc(
        "TRN2", target_bir_lowering=False, debug=False, num_devices=n_cores
    )

    xs_d = nc.dram_tensor("xs", [XROWS, H], F16, kind="ExternalInput").ap()
    # pre-tiled on host: wg/wu [128, IC*HC*128] with [p, ic, hc, i] layout,
    # wd [128, HC*IC*128] with [p, hc, ic, h] layout.
    wg_d = nc.dram_tensor("wg", [P, IC * HC * P], F16, kind="ExternalInput").ap()
    wu_d = nc.dram_tensor("wu", [P, IC * HC * P], F16, kind="ExternalInput").ap()
    wd_d = nc.dram_tensor("wd", [P, HC * IC * P], F16, kind="ExternalInput").ap()
    out_d = nc.dram_tensor("out", [TS, H + 4], mybir.dt.int8,
                           kind="ExternalOutput").ap()

    with tile.TileContext(nc) as tc:
        with contextlib.ExitStack() as top:
            dram = top.enter_context(tc.tile_pool(name="dram", bufs=1, space="DRAM"))
            xloc_h = [dram.tile([TS, H // 2], F16, name=f"xloc{h}")
                      for h in range(2)]  # own token rows, column halves
            xfull_h = [dram.tile([n_cores * TS, H // 2], F16,
                                 addr_space="Shared", name=f"xfull{h}")
                       for h in range(2)]
            part_g = [dram.tile([T, HG], F16, name=f"part{g}") for g in range(NG)]
            rs_g = [dram.tile([TS, HG], F16, name=f"rs{g}") for g in range(NG)]

            const = top.enter_context(tc.tile_pool(name="const", bufs=1))
            identh = const.tile([P, P], F16)
            make_identity(nc, identh)
            # gather idx: int16, idx[i] at [i%16, i//16], replicated to all
            # 8 gpsimd-core partition stripes
            gidx_t = const.tile([P, NS], I16)
            for r in range(8):
                nc.sync.dma_start(
                    gidx_t[16 * r : 16 * (r + 1), :],
                    xs_d[TS : TS + 1, 0:CG].bitcast(I16).rearrange(
                        "r (p s) -> p (r s)", p=16
                    ),
                )
            # scatter idx: int32, idx[j*128+p] at [p, j]
            sidx_t = const.tile([P, NJP], I32)
            nc.sync.dma_start(
                sidx_t,
                xs_d[TS + 1 : TS + 2, 0 : 2 * NJP * P].bitcast(I32).rearrange(
                    "r (p j) -> p (r j)", p=P
                ),
            )
            wvh = const.tile([P, NJP], F16)
            nc.sync.dma_start(
                wvh,
                xs_d[TS + 2 : TS + 3, 0 : NJP * P].rearrange(
                    "r (p j) -> p (r j)", p=P
                ),
            )
            wvl = const.tile([P, NJP], F16)
            nc.sync.dma_start(
                wvl,
                xs_d[TS + 3 : TS + 4, 0 : NJP * P].rearrange(
                    "r (p j) -> p (r j)", p=P
                ),
            )
            wv = const.tile([P, NJP], F32)
            wvlo = const.tile([P, NJP], F32)
            nc.vector.tensor_copy(wv, wvh)
            nc.vector.tensor_copy(wvlo, wvl)
            nc.vector.tensor_add(wv, wv, wvlo)

            # ---- phase 0: AllGather token-major x in two column halves ----
            # (the second half's collective and gather overlap the first
            # stage-1 half-pass on the PE)
            HH = HC // 2
            for h in range(2):
                nc.sync.dma_start(
                    xloc_h[h][:], xs_d[0:TS, h * (H // 2) : (h + 1) * (H // 2)]
                )
            for h in range(2):
                nc.gpsimd.collective_compute(
                    "AllGather",
                    ALU.bypass,
                    replica_groups=[list(range(n_cores))],
                    ins=[xloc_h[h][:].opt()],
                    outs=[xfull_h[h][:].opt()],
                )

            mid = top.enter_context(contextlib.ExitStack())
            mp = mid.enter_context(tc.tile_pool(name="mid", bufs=1))
            xgT = mp.tile([P, HC, CG], F16)  # x^T for my tokens (+gather pad)
            act = mp.tile([P, IC, C], F16)  # silu(g)*u

            # ---- phase 1: transpose-mode gathers, one per column half ------
            for h in range(2):
                nc.gpsimd.dma_gather(
                    out_ap=xgT[:, h * HH : (h + 1) * HH, :],
                    in_ap=xfull_h[h][:],
                    idxs_ap=gidx_t[:],
                    num_idxs=CG,
                    num_idxs_reg=CG,
                    elem_size=H // 2,
                    transpose=True,
                )

            # zero the partial-output scratch (rows not scattered must be 0);
            # emitted after the gathers so these DMAs don't compete with the
            # AG-critical path
            zrow = const.tile([P, H], F16)
            nc.vector.memset(zrow, 0.0)
            for g in range(NG):
                for tt in range(T // P):
                    nc.sync.dma_start(
                        part_g[g][tt * P : (tt + 1) * P, :], zrow[:, 0:HG]
                    )

            # ---- phase 2: stage 1 (gate/up + SwiGLU) on C tokens -----------
            # The first B1 ic rows run as two half-contractions: the h-low
            # half-pass only needs the first AG/gather half, so the PE works
            # while the second half is still arriving. Partial sums stage to
            # SBUF fp16 and are added back in the h-high pass.
            B1 = min(28, IC)
            with contextlib.ExitStack() as ph:
                w1p = ph.enter_context(tc.tile_pool(name="w1p", bufs=3))
                sp = ph.enter_context(tc.tile_pool(name="sp", bufs=2))
                hp = ph.enter_context(tc.tile_pool(name="hp", bufs=1))
                s1ps = ph.enter_context(
                    tc.tile_pool(name="s1ps", bufs=2, space="PSUM")
                )
                gh = hp.tile([P, B1, C], F16)
                uh = hp.tile([P, B1, C], F16)
                # pass A: ic < B1, h-blocks 0..HH-1 only
                for ic in range(B1):
                    wgt = w1p.tile([P, HH * P], F16, tag="wga")
                    nc.sync.dma_start(
                        wgt, wg_d[:, ic * HC * P : (ic * HC + HH) * P]
                    )
                    wut = w1p.tile([P, HH * P], F16, tag="wua")
                    nc.sync.dma_start(
                        wut, wu_d[:, ic * HC * P : (ic * HC + HH) * P]
                    )
                    pgs = [
                        s1ps.tile([P, e - s], F32, tag=f"pg{k}",
                                  name=f"pga{k}_{ic}")
                        for k, (s, e) in enumerate(chunks)
                    ]
                    pus = [
                        s1ps.tile([P, e - s], F32, tag=f"pu{k}",
                                  name=f"pua{k}_{ic}")
                        for k, (s, e) in enumerate(chunks)
                    ]
                    for hc in range(HH):
                        lg_ = wgt[:, hc * P : (hc + 1) * P]
                        lu_ = wut[:, hc * P : (hc + 1) * P]
                        for k, (s, e) in enumerate(chunks):
                            nc.tensor.matmul(
                                pgs[k], lhsT=lg_, rhs=xgT[:, hc, s:e],
                                start=(hc == 0), stop=(hc == HH - 1),
                            )
                        for k, (s, e) in enumerate(chunks):
                            nc.tensor.matmul(
                                pus[k], lhsT=lu_, rhs=xgT[:, hc, s:e],
                                start=(hc == 0), stop=(hc == HH - 1),
                            )
                    for k, (s, e) in enumerate(chunks):
                        nc.vector.tensor_copy(gh[:, ic, s:e], pgs[k])
                        nc.vector.tensor_copy(uh[:, ic, s:e], pus[k])
                # pass B: ic < B1, h-blocks HH..HC-1, combine + SwiGLU
                for ic in range(B1):
                    wgt = w1p.tile([P, HH * P], F16, tag="wgb")
                    nc.sync.dma_start(
                        wgt, wg_d[:, (ic * HC + HH) * P : (ic + 1) * HC * P]
                    )
                    wut = w1p.tile([P, HH * P], F16, tag="wub")
                    nc.sync.dma_start(
                        wut, wu_d[:, (ic * HC + HH) * P : (ic + 1) * HC * P]
                    )
                    pgs = [
                        s1ps.tile([P, e - s], F32, tag=f"pg{k}",
                                  name=f"pgb{k}_{ic}")
                        for k, (s, e) in enumerate(chunks)
                    ]
                    pus = [
                        s1ps.tile([P, e - s], F32, tag=f"pu{k}",
                                  name=f"pub{k}_{ic}")
                        for k, (s, e) in enumerate(chunks)
                    ]
                    for hc in range(HH, HC):
                        lg_ = wgt[:, (hc - HH) * P : (hc - HH + 1) * P]
                        lu_ = wut[:, (hc - HH) * P : (hc - HH + 1) * P]
                        for k, (s, e) in enumerate(chunks):
                            nc.tensor.matmul(
                                pgs[k], lhsT=lg_, rhs=xgT[:, hc, s:e],
                                start=(hc == HH), stop=(hc == HC - 1),
                            )
                        for k, (s, e) in enumerate(chunks):
                            nc.tensor.matmul(
                                pus[k], lhsT=lu_, rhs=xgT[:, hc, s:e],
                                start=(hc == HH), stop=(hc == HC - 1),
                            )
                    for k, (s, e) in enumerate(chunks):
                        gt = sp.tile([P, e - s], F32, tag=f"gt{k}",
                                     name=f"gt{k}_{ic}")
                        nc.vector.tensor_tensor(
                            gt, pgs[k], gh[:, ic, s:e], op=ALU.add
                        )
                        sig = sp.tile([P, e - s], F32, tag=f"sig{k}",
                                      name=f"sigb{k}_{ic}")
                        nc.scalar.activation(sig, gt, AF.Sigmoid)
                        nc.vector.tensor_mul(sig, sig, gt)
                        ut = sp.tile([P, e - s], F32, tag=f"ut{k}",
                                     name=f"ut{k}_{ic}")
                        nc.vector.tensor_tensor(
                            ut, pus[k], uh[:, ic, s:e], op=ALU.add
                        )
                        nc.vector.tensor_tensor(
                            act[:, ic, s:e], sig, ut, op=ALU.mult
                        )
                # remaining ics: normal single-pass over all 16 h-blocks
                for ic in range(B1, IC):
                    wgt = w1p.tile([P, HC * P], F16, tag="wg")
                    nc.sync.dma_start(
                        wgt, wg_d[:, ic * HC * P : (ic + 1) * HC * P]
                    )
                    wut = w1p.tile([P, HC * P], F16, tag="wu")
                    nc.sync.dma_start(
                        wut, wu_d[:, ic * HC * P : (ic + 1) * HC * P]
                    )
                    pgs = [
                        s1ps.tile([P, e - s], F32, tag=f"pg{k}",
                                  name=f"pg{k}_{ic}")
                        for k, (s, e) in enumerate(chunks)
                    ]
                    pus = [
                        s1ps.tile([P, e - s], F32, tag=f"pu{k}",
                                  name=f"pu{k}_{ic}")
                        for k, (s, e) in enumerate(chunks)
                    ]
                    for hc in range(HC):
                        lg_ = wgt[:, hc * P : (hc + 1) * P]
                        lu_ = wut[:, hc * P : (hc + 1) * P]
                        for k, (s, e) in enumerate(chunks):
                            nc.tensor.matmul(
                                pgs[k], lhsT=lg_, rhs=xgT[:, hc, s:e],
                                start=(hc == 0), stop=(hc == HC - 1),
                            )
                        for k, (s, e) in enumerate(chunks):
                            nc.tensor.matmul(
                                pus[k], lhsT=lu_, rhs=xgT[:, hc, s:e],
                                start=(hc == 0), stop=(hc == HC - 1),
                            )
                    for k, (s, e) in enumerate(chunks):
                        sig = sp.tile([P, e - s], F32, tag=f"sig{k}",
                                      name=f"sig{k}_{ic}")
                        nc.scalar.activation(sig, pgs[k], AF.Sigmoid)
                        nc.vector.tensor_mul(sig, sig, pgs[k])
                        nc.vector.tensor_tensor(
                            act[:, ic, s:e], sig, pus[k], op=ALU.mult
                        )

            # ---- phase 3: stage 2 per column group; RS overlaps next group -
            with contextlib.ExitStack() as ph:
                w2p = ph.enter_context(tc.tile_pool(name="w2p", bufs=2))
                yp = ph.enter_context(tc.tile_pool(name="yp", bufs=2))
                ymp = ph.enter_context(tc.tile_pool(name="ymp", bufs=1))
                s2ps = ph.enter_context(
                    tc.tile_pool(name="s2ps", bufs=2, space="PSUM")
                )
                t2ps = ph.enter_context(
                    tc.tile_pool(name="t2ps", bufs=2, space="PSUM")
                )
                ytm = ymp.tile([P, NJP, H], F16)  # token-major scaled y
                for g in range(NG):
                    for hg in range(HCG):
                        hc = g * HCG + hg
                        wdt = w2p.tile([P, IC * P], F16, tag="wd")
                        nc.sync.dma_start(
                            wdt, wd_d[:, hc * IC * P : (hc + 1) * IC * P]
                        )
                        pys = [
                            s2ps.tile([P, e - s], F32, tag=f"py{k}",
                                      name=f"py{k}_{hc}")
                            for k, (s, e) in enumerate(chunks)
                        ]
                        for ic in range(IC):
                            ld_ = wdt[:, ic * P : (ic + 1) * P]
                            for k, (s, e) in enumerate(chunks):
                                nc.tensor.matmul(
                                    pys[k], lhsT=ld_, rhs=act[:, ic, s:e],
                                    start=(ic == 0), stop=(ic == IC - 1),
                                )
                        yts = yp.tile([P, C], F16, tag="yts")
                        for k, (s, e) in enumerate(chunks):
                            nc.vector.tensor_copy(yts[:, s:e], pys[k])
                        for j in range(NJP):
                            w_ = P if j < NJP - 1 else PL
                            tp = t2ps.tile([P, P], F16, tag="ytp")
                            nc.tensor.transpose(
                                tp[0:w_, :], yts[:, j * P : j * P + w_],
                                identh,
                            )
                            nc.vector.tensor_scalar(
                                ytm[0:w_, j, hc * P : (hc + 1) * P],
                                tp[0:w_, :], wv[0:w_, j : j + 1], None,
                                op0=ALU.mult,
                            )
                    for j in range(NJP):
                        w_ = P if j < NJP - 1 else PL
                        nc.gpsimd.indirect_dma_start(
                            out=part_g[g][:],
                            out_offset=bass.IndirectOffsetOnAxis(
                                ap=sidx_t[0:w_, j : j + 1], axis=0
                            ),
                            in_=ytm[0:w_, j, g * HG : (g + 1) * HG],
                            in_offset=None,
                            bounds_check=T - 1,
                            oob_is_err=False,
                        )
                    nc.gpsimd.collective_compute(
                        "ReduceScatter",
                        ALU.add,
                        replica_groups=[list(range(n_cores))],
                        ins=[part_g[g][:].opt()],
                        outs=[rs_g[g][:].opt()],
                    )

            mid.close()  # free xgT/act before the tail

            # per-token symmetric int8 quantization for the return trip
            with contextlib.ExitStack() as ph:
                op_ = ph.enter_context(tc.tile_pool(name="outp", bufs=2))
                for st in range(TS // P):
                    of = op_.tile([P, H], F32, tag="of")
                    for g in range(NG):
                        ofb = op_.tile([P, HG], F16, tag=f"ofb{g}",
                                       name=f"ofb{g}_{st}")
                        nc.sync.dma_start(
                            ofb, rs_g[g][st * P : (st + 1) * P, :]
                        )
                        nc.vector.tensor_copy(
                            of[:, g * HG : (g + 1) * HG], ofb
                        )
                    ab = op_.tile([P, H], F32, tag="ab")
                    nc.scalar.activation(ab, of, AF.Abs)
                    mx = op_.tile([P, 1], F32, tag="mx")
                    nc.vector.reduce_max(mx, ab, axis=AX.X)
                    nc.vector.tensor_scalar_add(mx, mx, 1e-30)
                    inv = op_.tile([P, 1], F32, tag="inv")
                    nc.vector.reciprocal(inv, mx)
                    nc.vector.tensor_scalar(inv, inv, 127.0, None, op0=ALU.mult)
                    q = op_.tile([P, H], F32, tag="q")
                    nc.vector.tensor_scalar(q, of, inv, None, op0=ALU.mult)
                    qi = op_.tile([P, H], mybir.dt.int8, tag="qi")
                    nc.vector.tensor_copy(qi, q)
                    nc.sync.dma_start(out_d[st * P : (st + 1) * P, 0:H], qi)
                    sc = op_.tile([P, 1], F32, tag="sc")
                    nc.vector.tensor_scalar(
                        sc, mx, 1.0 / 127.0, None, op0=ALU.mult
                    )
                    nc.sync.dma_start(
                        out_d[st * P : (st + 1) * P, H : H + 4].bitcast(F32), sc
                    )

    nc.compile()
    return nc


# ---------------------------------------------------------------------------
# dispatch: jit once, keep weights device-resident across calls


def _fingerprint(a: np.ndarray) -> bytes:
    h = hashlib.blake2b(digest_size=16)
    h.update(repr((a.shape, str(a.dtype))).encode())
    b = a.reshape(-1)
    step = max(1, b.size // 262144)
    h.update(np.ascontiguousarray(b[::step]).tobytes())
    return h.digest()


class _State:
    def __init__(self, C):
        install_neuronx_cc_hook()
        self.C = C
        self.nc = build_moe(C)
        nc = self.nc
        devices = jax.devices()[:NCORES]
        assert len(devices) == NCORES, f"need {NCORES} devices"
        self.mesh = Mesh(np.asarray(devices), ("core",))
        self.sharding = NamedSharding(self.mesh, PartitionSpec("core"))

        in_names, in_avals, out_names, out_avals = [], [], [], []
        pname = nc.partition_id_tensor.name if nc.partition_id_tensor else None
        for alloc in nc.m.functions[0].allocations:
            if not isinstance(alloc, mybir.MemoryLocationSet):
                continue
            name = alloc.memorylocations[0].name
            if alloc.kind == "ExternalInput":
                if name != pname:
                    in_names.append(name)
                    in_avals.append((tuple(alloc.tensor_shape),
                                     mybir.dt.np(alloc.dtype)))
            elif alloc.kind == "ExternalOutput":
                out_names.append(name)
                out_avals.append(
                    jax.core.ShapedArray(
                        tuple(alloc.tensor_shape), mybir.dt.np(alloc.dtype)
                    )
                )
        self.in_names = in_names
        bind_names = tuple(in_names) + ((pname,) if pname else ())
        out_avals = tuple(out_avals)
        out_names = tuple(out_names)

        def _body(*args):
            ops = list(args)
            if pname:
                ops.append(partition_id_tensor())
            outs = _bass_exec_p.bind(
                *ops,
                out_avals=out_avals,
                in_names=bind_names,
                out_names=out_names,
                lowering_input_output_aliases=(),
                sim_require_finite=True,
                sim_require_nnan=True,
                nc=nc,
            )
            return tuple(outs)

        n_in = len(in_names)
        sm = shard_map(
            _body,
            mesh=self.mesh,
            in_specs=(PartitionSpec("core"),) * n_in,
            out_specs=(PartitionSpec("core"),),
            check_rep=False,
        )

        def compile_fn():
            jt = jax.jit(sm, keep_unused=True)
            args = [
                jax.ShapeDtypeStruct(
                    (NCORES * shape[0],) + tuple(shape[1:]), dt,
                    sharding=self.sharding,
                )
                for shape, dt in in_avals
            ]
            return jt.lower(*args).compile()

        self.jitted = fast_dispatch_compile(compile_fn)
        self._wcache = {}  # name -> (src_ref, fingerprint, device_array)

    def _cached(self, name, src, prep):
        ent = self._wcache.get(name)
        if ent is not None and ent[0] is src:
            return ent[2]
        fp = _fingerprint(src)
        if ent is not None and ent[1] == fp:
            self._wcache[name] = (src, fp, ent[2])
            return ent[2]
        arr = jax.device_put(prep(src), self.sharding)
        self._wcache[name] = (src, fp, arr)
        return arr

    def weights(self, w_gate, w_up, w_down):
        IC, HC = I0 // P, H0 // P

        def prep_1(w):  # [E, I, H] -> concat_e [128, IC*HC*128], [p,ic,hc,i]
            w = np.asarray(w, np.float32).astype(NP16)
            parts = [
                np.ascontiguousarray(
                    w[e].reshape(IC, P, HC, P).transpose(3, 0, 2, 1)
                ).reshape(P, IC * HC * P)
                for e in range(NCORES)
            ]
            return np.concatenate(parts, axis=0)

        def prep_2(w):  # [E, H, I] -> concat_e [128, HC*IC*128], [p,hc,ic,h]
            w = np.asarray(w, np.float32).astype(NP16)
            parts = [
                np.ascontiguousarray(
                    w[e].reshape(HC, P, IC, P).transpose(3, 0, 2, 1)
                ).reshape(P, HC * IC * P)
                for e in range(NCORES)
            ]
            return np.concatenate(parts, axis=0)

        return {
            "wg": self._cached("wg", w_gate, prep_1),
            "wu": self._cached("wu", w_up, prep_1),
            "wd": self._cached("wd", w_down, prep_2),
        }


_STATE = None


def _get_state(C=576):
    global _STATE
    if _STATE is None or _STATE.C < C:
        _STATE = _State(C)
    return _STATE


def _host_router(x, w_router):
    """Exact fp32 top-2 router. Returns (token lists, combine weights) per
    expert: lists[e] sorted token ids, cw[e] the matching softmax weights."""
    logits = x @ np.asarray(w_router, np.float32).T  # [T, E]
    i1 = np.argmax(logits, axis=1)
    v1 = np.take_along_axis(logits, i1[:, None], axis=1)[:, 0]
    masked = logits.copy()
    np.put_along_axis(masked, i1[:, None], -np.inf, axis=1)
    i2 = np.argmax(masked, axis=1)
    v2 = np.take_along_axis(masked, i2[:, None], axis=1)[:, 0]
    e = np.exp(v2 - v1)
    w1 = 1.0 / (1.0 + e)
    w2 = e * w1
    T, E = logits.shape
    lists, cws = [], []
    for ei in range(E):
        t1 = np.nonzero(i1 == ei)[0]
        t2 = np.nonzero(i2 == ei)[0]
        tok = np.concatenate([t1, t2])
        w = np.concatenate([w1[t1], w2[t2]])
        order = np.argsort(tok, kind="stable")
        lists.append(tok[order].astype(np.int32))
        cws.append(w[order].astype(np.float32))
    return lists, cws


_PACK_BUF = None
_PACK_POOL = None
_DEC_POOL = None


def _pack_xs(x, w_router, C):
    """[NCORES*XROWS, H] fp16: per core, its x shard plus aux rows holding
    the gather idx (int16), scatter idx (int32) and combine-weight hi/lo."""
    global _PACK_BUF, _PACK_POOL
    from concurrent.futures import ThreadPoolExecutor

    T, H = x.shape
    NJP = (C + P - 1) // P
    SLOTS = NJP * P
    if _PACK_BUF is None or _PACK_BUF.shape != (NCORES * XROWS, H):
        _PACK_BUF = np.zeros((NCORES * XROWS, H), NP16)
    if _PACK_POOL is None:
        _PACK_POOL = ThreadPoolExecutor(1)
    a = _PACK_BUF

    def _cast():
        for c in range(NCORES):
            a[c * XROWS : c * XROWS + TS, :] = x[c * TS : (c + 1) * TS]

    fut = _PACK_POOL.submit(_cast)
    lists, cws = _host_router(x, w_router)  # overlaps _cast
    maxload = max(len(l) for l in lists)
    assert maxload <= C, f"expert load {maxload} exceeds capacity {C}"
    fut.result()
    for c in range(NCORES):
        n = len(lists[c])
        gidx = np.zeros(SLOTS, np.int16)  # pads gather row 0 (cw=0 kills)
        gidx[:n] = lists[c].astype(np.int16)
        sidx = np.full(SLOTS, PAD_IDX, np.int32)  # pads skipped by bounds
        sidx[:n] = lists[c]
        cw = np.zeros(SLOTS, np.float32)
        cw[:n] = cws[c]
        r = c * XROWS + TS
        # gather idx: device reads [16, S//16] with idx[i] at [i%16, i//16]
        a[r, :SLOTS] = gidx.reshape(SLOTS // 16, 16).T.reshape(-1).view(NP16)
        a[r, SLOTS:] = 0
        # scatter idx + cw: device reads [128, NJP] with elem p*NJP+j <-
        # slot[j*128+p]
        sperm = sidx.reshape(NJP, P).T.reshape(-1)
        cwp = cw.reshape(NJP, P).T.reshape(-1)
        hi = cwp.astype(NP16)
        lo = (cwp - hi.astype(np.float32)).astype(NP16)
        a[r + 1, : 2 * SLOTS] = sperm.view(NP16)
        a[r + 1, 2 * SLOTS :] = 0
        a[r + 2, :SLOTS] = hi
        a[r + 2, SLOTS:] = 0
        a[r + 3, :SLOTS] = lo
        a[r + 3, SLOTS:] = 0
    return a, maxload


def kernel(x, w_router, w_gate, w_up, w_down, top_k):
    try:
        return _kernel_impl(x, w_router, w_gate, w_up, w_down, top_k)
    except AssertionError:
        raise
    except Exception:
        # transient device failures (e.g. NRT_EXEC_UNIT_UNRECOVERABLE) have
        # been observed on this fabric; rebuild the backend + state and
        # retry once. Any failure inside the recovery path re-raises.
        global _STATE
        _STATE = None
        try:
            import jax.extend.backend as _jeb

            _jeb.clear_backends()
        except Exception:
            pass
        try:
            jax.clear_caches()
        except Exception:
            pass
        return _kernel_impl(x, w_router, w_gate, w_up, w_down, top_k)


def _kernel_impl(x, w_router, w_gate, w_up, w_down, top_k):
    import time as _time

    t0 = _time.time()
    assert int(top_k) == 2, f"kernel specialized for top_k=2, got {top_k}"
    x = np.ascontiguousarray(np.asarray(x, dtype=np.float32))
    w_router = np.asarray(w_router)
    w_gate, w_up, w_down = (np.asarray(a) for a in (w_gate, w_up, w_down))
    T, H = x.shape
    E, I = w_gate.shape[0], w_gate.shape[1]
    assert (T, H, I, E) == (T0, H0, I0, E0), "kernel hardcoded for spec shapes"

    st = _get_state()
    try:
        packed, maxload = _pack_xs(x, w_router, st.C)
    except AssertionError:
        # an input whose max expert load exceeds capacity: rebuild bigger
        lists, _cw = _host_router(x, np.asarray(w_router, np.float32))
        ml = max(len(l) for l in lists)
        st = _get_state(((ml + 15) // 16) * 16)
        packed, maxload = _pack_xs(x, w_router, st.C)
    ws = st.weights(w_gate, w_up, w_down)
    xg = jax.device_put(packed, st.sharding)
    args = {"xs": xg, **ws}
    (out,) = st.jitted(*[args[n] for n in st.in_names])
    buf = np.asarray(out)  # int8 [T, H+4]
    scale = buf[:, H : H + 4].copy().view(np.float32)  # [T, 1]
    global _DEC_POOL
    if _DEC_POOL is None:
        from concurrent.futures import ThreadPoolExecutor

        _DEC_POOL = ThreadPoolExecutor(NCORES)
    res = np.empty((T, H), np.float32)
    rb = T // NCORES

    def _dec(b):
        r0, r1 = b * rb, (b + 1) * rb
        np.multiply(buf[r0:r1, :H], scale[r0:r1], dtype=np.float32,
                    out=res[r0:r1])

    list(_DEC_POOL.map(_dec, range(NCORES)))
    kernel._last_wall_s = _time.time() - t0
    kernel._last_exec_time_ns = None
    return res


def device_args(x, w_router, w_gate, w_up, w_down):
    """Device-resident inputs for steady-state benchmarking."""
    st = _get_state()
    packed, _ = _pack_xs(np.ascontiguousarray(np.asarray(x, np.float32)),
                         w_router, st.C)
    ws = st.weights(w_gate, w_up, w_down)
    xg = jax.device_put(packed, st.sharding)
    args = {"xs": xg, **ws}
    return st, [args[n] for n in st.in_names]


# revision 16
# speedup vs baseline: 1.4053x; 1.3518x over previous
"""Sparse expert-parallel MoE (top-2 of 8 experts, SwiGLU) for 8 TRN2 cores.

Core e holds expert e's weights in fp16 (pre-tiled on host for contiguous
DMA). The top-2 router runs on the host in exact fp32, so routing decisions
match the reference bit-for-bit; each core receives the sorted token-id list
routed to its expert (capacity C=576 >= max load, rebuilt bigger if an
input ever exceeds it) plus per-token combine
weights as an fp16 hi/lo pair. Pad slots gather row 0 (combine weight 0) and
use an out-of-bounds scatter index that the indirect DMA skips.

Per call, each core (one SPMD program):
  1. AllGathers the [T/8, H] fp16 token shards (token-major) so every core
     has all T rows, then one transpose-mode dma_gather pulls its expert's C
     token rows directly into [H, C] transposed layout in SBUF.
  2. SwiGLU FFN over only its C tokens (fp16 matmuls, fp32 psum): stage 1
     keeps silu(g)*u in SBUF; stage 2 streams w_down in two column groups,
     transposes y back to token-major on the PE and scales rows by the
     combine weight.
  3. Scatters the scaled rows into a zeroed [T, H/NG] fp16 partial per
     column group via indirect DMA; each group ReduceScatters as soon as it
     is complete, overlapping the collective with the next group's matmuls.
     The [T/8, H] shard returns as per-token-scaled int8 (scale embedded as
     4 extra bytes per row).

Dispatch uses bass2jax fast_dispatch_compile (C++ fast path). Weights are
uploaded once as committed sharded jax.Arrays; warm calls move only the
token activations in and the int8 shards back.
"""

import contextlib
import hashlib
import sys

import numpy as np

sys.path.insert(0, "/opt/trn_rl_repo")

import jax  # noqa: E402
from jax.sharding import Mesh, NamedSharding, PartitionSpec  # noqa: E402

from concourse import bacc, bass, mybir, tile  # noqa: E402
from concourse.bass2jax import (  # noqa: E402
    _bass_exec_p,
    fast_dispatch_compile,
    install_neuronx_cc_hook,
    partition_id_tensor,
)
from concourse.masks import make_identity  # noqa: E402
from jax.experimental.shard_map import shard_map  # noqa: E402

F32 = mybir.dt.float32
F16 = mybir.dt.float16
I32 = mybir.dt.int32
I16 = mybir.dt.int16
AF = mybir.ActivationFunctionType
ALU = mybir.AluOpType
AX = mybir.AxisListType

P = 128
NCORES = 8
T0, H0, I0, E0 = 2048, 2048, 5632, 8
TS = T0 // NCORES  # 256 tokens per shard
XROWS = TS + 8  # shard rows + aux rows (gather idx, scatter idx, cw hi/lo)
NP16 = np.float16
PAD_IDX = 1 << 20  # scatter pad: > bounds_check => row skipped
NG = 2  # ReduceScatter column groups (H/NG columns each)


def build_moe(C, n_cores=NCORES):
    """Sparse expert-parallel SPMD Bass program; C = token capacity/expert."""
    T, H, I = T0, H0, I0
    HC = H // P  # 16
    IC = I // P  # 44
    NJP = (C + P - 1) // P  # token tiles (last may be partial)
    PL = C - (NJP - 1) * P  # rows in the last tile
    CG = NJP * P  # gather width (dma_gather needs a multiple of 128)
    NS = CG // 16  # int16 idx columns
    chunks = [(s, min(s + 512, C)) for s in range(0, C, 512)]
    HG = H // NG  # columns per RS group
    HCG = HC // NG  # h-blocks per RS group

    nc = bacc.Bacc(
        "TRN2", target_bir_lowering=False, debug=False, num_devices=n_cores
    )

    xs_d = nc.dram_tensor("xs", [XROWS, H], F16, kind="ExternalInput").ap()
    # pre-tiled on host: wg/wu [128, IC*HC*128] with [p, ic, hc, i] layout,
    # wd [128, HC*IC*128] with [p, hc, ic, h] layout.
    wg_d = nc.dram_tensor("wg", [P, IC * HC * P], F16, kind="ExternalInput").ap()
    wu_d = nc.dram_tensor("wu", [P, IC * HC * P], F16, kind="ExternalInput").ap()
    wd_d = nc.dram_tensor("wd", [P, HC * IC * P], F16, kind="ExternalInput").ap()
    out_d = nc.dram_tensor("out", [TS, H + 4], mybir.dt.int8,
                           kind="ExternalOutput").ap()

    with tile.TileContext(nc) as tc:
        with contextlib.ExitStack() as top:
            dram = top.enter_context(tc.tile_pool(name="dram", bufs=1, space="DRAM"))
            xloc_h = [dram.tile([TS, H // 2], F16, name=f"xloc{h}")
                      for h in range(2)]  # own token rows, column halves
            xfull_h = [dram.tile([n_cores * TS, H // 2], F16,
                                 addr_space="Shared", name=f"xfull{h}")
                       for h in range(2)]
            part_g = [dram.tile([T, HG], F16, name=f"part{g}") for g in range(NG)]
            rs_g = [dram.tile([TS, HG], F16, name=f"rs{g}") for g in range(NG)]

            const = top.enter_context(tc.tile_pool(name="const", bufs=1))
            identh = const.tile([P, P], F16)
            make_identity(nc, identh)
            # gather idx: int16, idx[i] at [i%16, i//16], replicated to all
            # 8 gpsimd-core partition stripes
            gidx_t = const.tile([P, NS], I16)
            for r in range(8):
                nc.sync.dma_start(
                    gidx_t[16 * r : 16 * (r + 1), :],
                    xs_d[TS : TS + 1, 0:CG].bitcast(I16).rearrange(
                        "r (p s) -> p (r s)", p=16
                    ),
                )
            # scatter idx: int32, idx[j*128+p] at [p, j]
            sidx_t = const.tile([P, NJP], I32)
            nc.sync.dma_start(
                sidx_t,
                xs_d[TS + 1 : TS + 2, 0 : 2 * NJP * P].bitcast(I32).rearrange(
                    "r (p j) -> p (r j)", p=P
                ),
            )
            wvh = const.tile([P, NJP], F16)
            nc.sync.dma_start(
                wvh,
                xs_d[TS + 2 : TS + 3, 0 : NJP * P].rearrange(
                    "r (p j) -> p (r j)", p=P
                ),
            )
            wvl = const.tile([P, NJP], F16)
            nc.sync.dma_start(
                wvl,
                xs_d[TS + 3 : TS + 4, 0 : NJP * P].rearrange(
                    "r (p j) -> p (r j)", p=P
                ),
            )
            wv = const.tile([P, NJP], F32)
            wvlo = const.tile([P, NJP], F32)
            nc.vector.tensor_copy(wv, wvh)
            nc.vector.tensor_copy(wvlo, wvl)
            nc.vector.tensor_add(wv, wv, wvlo)

            # ---- phase 0: AllGather token-major x in two column halves ----
            # (the second half's collective and gather overlap the first
            # stage-1 half-pass on the PE)
            HH = HC // 2
            for h in range(2):
                nc.sync.dma_start(
                    xloc_h[h][:], xs_d[0:TS, h * (H // 2) : (h + 1) * (H // 2)]
                )
            for h in range(2):
                nc.gpsimd.collective_compute(
                    "AllGather",
                    ALU.bypass,
                    replica_groups=[list(range(n_cores))],
                    ins=[xloc_h[h][:].opt()],
                    outs=[xfull_h[h][:].opt()],
                )

            mid = top.enter_context(contextlib.ExitStack())
            mp = mid.enter_context(tc.tile_pool(name="mid", bufs=1))
            xgT = mp.tile([P, HC, CG], F16)  # x^T for my tokens (+gather pad)
            act = mp.tile([P, IC, C], F16)  # silu(g)*u

            # ---- phase 1: transpose-mode gathers, one per column half ------
            for h in range(2):
                nc.gpsimd.dma_gather(
                    out_ap=xgT[:, h * HH : (h + 1) * HH, :],
                    in_ap=xfull_h[h][:],
                    idxs_ap=gidx_t[:],
                    num_idxs=CG,
                    num_idxs_reg=CG,
                    elem_size=H // 2,
                    transpose=True,
                )

            # zero the partial-output scratch (rows not scattered must be 0);
            # emitted after the gathers so these DMAs don't compete with the
            # AG-critical path
            zrow = const.tile([P, H], F16)
            nc.vector.memset(zrow, 0.0)
            for g in range(NG):
                for tt in range(T // P):
                    nc.sync.dma_start(
                        part_g[g][tt * P : (tt + 1) * P, :], zrow[:, 0:HG]
                    )

            # ---- phase 2: stage 1 (gate/up + SwiGLU) on C tokens -----------
            # The first B1 ic rows run as two half-contractions: the h-low
            # half-pass only needs the first AG/gather half, so the PE works
            # while the second half is still arriving. Partial sums stage to
            # SBUF fp16 and are added back in the h-high pass.
            B1 = min(28, IC)
            with contextlib.ExitStack() as ph:
                w1p = ph.enter_context(tc.tile_pool(name="w1p", bufs=3))
                sp = ph.enter_context(tc.tile_pool(name="sp", bufs=2))
                hp = ph.enter_context(tc.tile_pool(name="hp", bufs=1))
                s1ps = ph.enter_context(
                    tc.tile_pool(name="s1ps", bufs=2, space="PSUM")
                )
                gh = hp.tile([P, B1, C], F16)
                uh = hp.tile([P, B1, C], F16)
                # pass A: ic < B1, h-blocks 0..HH-1 only
                for ic in range(B1):
                    wgt = w1p.tile([P, HH * P], F16, tag="wga")
                    nc.sync.dma_start(
                        wgt, wg_d[:, ic * HC * P : (ic * HC + HH) * P]
                    )
                    wut = w1p.tile([P, HH * P], F16, tag="wua")
                    nc.sync.dma_start(
                        wut, wu_d[:, ic * HC * P : (ic * HC + HH) * P]
                    )
                    pgs = [
                        s1ps.tile([P, e - s], F32, tag=f"pg{k}",
                                  name=f"pga{k}_{ic}")
                        for k, (s, e) in enumerate(chunks)
                    ]
                    pus = [
                        s1ps.tile([P, e - s], F32, tag=f"pu{k}",
                                  name=f"pua{k}_{ic}")
                        for k, (s, e) in enumerate(chunks)
                    ]
                    for hc in range(HH):
                        lg_ = wgt[:, hc * P : (hc + 1) * P]
                        lu_ = wut[:, hc * P : (hc + 1) * P]
                        for k, (s, e) in enumerate(chunks):
                            nc.tensor.matmul(
                                pgs[k], lhsT=lg_, rhs=xgT[:, hc, s:e],
                                start=(hc == 0), stop=(hc == HH - 1),
                            )
                        for k, (s, e) in enumerate(chunks):
                            nc.tensor.matmul(
                                pus[k], lhsT=lu_, rhs=xgT[:, hc, s:e],
                                start=(hc == 0), stop=(hc == HH - 1),
                            )
                    for k, (s, e) in enumerate(chunks):
                        nc.vector.tensor_copy(gh[:, ic, s:e], pgs[k])
                        nc.vector.tensor_copy(uh[:, ic, s:e], pus[k])
                # pass B: ic < B1, h-blocks HH..HC-1, combine + SwiGLU
                for ic in range(B1):
                    wgt = w1p.tile([P, HH * P], F16, tag="wgb")
                    nc.sync.dma_start(
                        wgt, wg_d[:, (ic * HC + HH) * P : (ic + 1) * HC * P]
                    )
                    wut = w1p.tile([P, HH * P], F16, tag="wub")
                    nc.sync.dma_start(
                        wut, wu_d[:, (ic * HC + HH) * P : (ic + 1) * HC * P]
                    )
                    pgs = [
                        s1ps.tile([P, e - s], F32, tag=f"pg{k}",
                                  name=f"pgb{k}_{ic}")
                        for k, (s, e) in enumerate(chunks)
                    ]
                    pus = [
                        s1ps.tile([P, e - s], F32, tag=f"pu{k}",
                                  name=f"pub{k}_{ic}")
                        for k, (s, e) in enumerate(chunks)
                    ]
                    for hc in range(HH, HC):
                        lg_ = wgt[:, (hc - HH) * P : (hc - HH + 1) * P]
                        lu_ = wut[:, (hc - HH) * P : (hc - HH + 1) * P]
                        for k, (s, e) in enumerate(chunks):
                            nc.tensor.matmul(
                                pgs[k], lhsT=lg_, rhs=xgT[:, hc, s:e],
                                start=(hc == HH), stop=(hc == HC - 1),
                            )
                        for k, (s, e) in enumerate(chunks):
                            nc.tensor.matmul(
                                pus[k], lhsT=lu_, rhs=xgT[:, hc, s:e],
                                start=(hc == HH), stop=(hc == HC - 1),
                            )
                    for k, (s, e) in enumerate(chunks):
                        gt = sp.tile([P, e - s], F32, tag=f"gt{k}",
                                     name=f"gt{k}_{ic}")
                        nc.vector.tensor_tensor(
                            gt, pgs[k], gh[:, ic, s:e], op=ALU.add
                        )
                        sig = sp.tile([P, e - s], F32, tag=f"sig{k}",
                                      name=f"sigb{k}_{ic}")
                        nc.scalar.activation(sig, gt, AF.Sigmoid)
                        nc.vector.tensor_mul(sig, sig, gt)
                        ut = sp.tile([P, e - s], F32, tag=f"ut{k}",
                                     name=f"ut{k}_{ic}")
                        nc.vector.tensor_tensor(
                            ut, pus[k], uh[:, ic, s:e], op=ALU.add
                        )
                        nc.vector.tensor_tensor(
                            act[:, ic, s:e], sig, ut, op=ALU.mult
                        )
                # remaining ics: normal single-pass over all 16 h-blocks
                for ic in range(B1, IC):
                    wgt = w1p.tile([P, HC * P], F16, tag="wg")
                    nc.sync.dma_start(
                        wgt, wg_d[:, ic * HC * P : (ic + 1) * HC * P]
                    )
                    wut = w1p.tile([P, HC * P], F16, tag="wu")
                    nc.sync.dma_start(
                        wut, wu_d[:, ic * HC * P : (ic + 1) * HC * P]
                    )
                    pgs = [
                        s1ps.tile([P, e - s], F32, tag=f"pg{k}",
                                  name=f"pg{k}_{ic}")
                        for k, (s, e) in enumerate(chunks)
                    ]
                    pus = [
                        s1ps.tile([P, e - s], F32, tag=f"pu{k}",
                                  name=f"pu{k}_{ic}")
                        for k, (s, e) in enumerate(chunks)
                    ]
                    for hc in range(HC):
                        lg_ = wgt[:, hc * P : (hc + 1) * P]
                        lu_ = wut[:, hc * P : (hc + 1) * P]
                        for k, (s, e) in enumerate(chunks):
                            nc.tensor.matmul(
                                pgs[k], lhsT=lg_, rhs=xgT[:, hc, s:e],
                                start=(hc == 0), stop=(hc == HC - 1),
                            )
                        for k, (s, e) in enumerate(chunks):
                            nc.tensor.matmul(
                                pus[k], lhsT=lu_, rhs=xgT[:, hc, s:e],
                                start=(hc == 0), stop=(hc == HC - 1),
                            )
                    for k, (s, e) in enumerate(chunks):
                        sig = sp.tile([P, e - s], F32, tag=f"sig{k}",
                                      name=f"sig{k}_{ic}")
                        nc.scalar.activation(sig, pgs[k], AF.Sigmoid)
                        nc.vector.tensor_mul(sig, sig, pgs[k])
                        nc.vector.tensor_tensor(
                            act[:, ic, s:e], sig, pus[k], op=ALU.mult
                        )

            # ---- phase 3: stage 2 per column group; RS overlaps next group -
            with contextlib.ExitStack() as ph:
                w2p = ph.enter_context(tc.tile_pool(name="w2p", bufs=2))
                yp = ph.enter_context(tc.tile_pool(name="yp", bufs=2))
                ymp = ph.enter_context(tc.tile_pool(name="ymp", bufs=1))
                s2ps = ph.enter_context(
                    tc.tile_pool(name="s2ps", bufs=2, space="PSUM")
                )
                t2ps = ph.enter_context(
                    tc.tile_pool(name="t2ps", bufs=2, space="PSUM")
                )
                ytm = ymp.tile([P, NJP, H], F16)  # token-major scaled y
                for g in range(NG):
                    for hg in range(HCG):
                        hc = g * HCG + hg
                        wdt = w2p.tile([P, IC * P], F16, tag="wd")
                        nc.sync.dma_start(
                            wdt, wd_d[:, hc * IC * P : (hc + 1) * IC * P]
                        )
                        pys = [
                            s2ps.tile([P, e - s], F32, tag=f"py{k}",
                                      name=f"py{k}_{hc}")
                            for k, (s, e) in enumerate(chunks)
                        ]
                        for ic in range(IC):
                            ld_ = wdt[:, ic * P : (ic + 1) * P]
                            for k, (s, e) in enumerate(chunks):
                                nc.tensor.matmul(
                                    pys[k], lhsT=ld_, rhs=act[:, ic, s:e],
                                    start=(ic == 0), stop=(ic == IC - 1),
                                )
                        yts = yp.tile([P, C], F16, tag="yts")
                        for k, (s, e) in enumerate(chunks):
                            nc.vector.tensor_copy(yts[:, s:e], pys[k])
                        for j in range(NJP):
                            w_ = P if j < NJP - 1 else PL
                            tp = t2ps.tile([P, P], F16, tag="ytp")
                            nc.tensor.transpose(
                                tp[0:w_, :], yts[:, j * P : j * P + w_],
                                identh,
                            )
                            nc.vector.tensor_scalar(
                                ytm[0:w_, j, hc * P : (hc + 1) * P],
                                tp[0:w_, :], wv[0:w_, j : j + 1], None,
                                op0=ALU.mult,
                            )
                    for j in range(NJP):
                        w_ = P if j < NJP - 1 else PL
                        nc.gpsimd.indirect_dma_start(
                            out=part_g[g][:],
                            out_offset=bass.IndirectOffsetOnAxis(
                                ap=sidx_t[0:w_, j : j + 1], axis=0
                            ),
                            in_=ytm[0:w_, j, g * HG : (g + 1) * HG],
                            in_offset=None,
                            bounds_check=T - 1,
                            oob_is_err=False,
                        )
                    nc.gpsimd.collective_compute(
                        "ReduceScatter",
                        ALU.add,
                        replica_groups=[list(range(n_cores))],
                        ins=[part_g[g][:].opt()],
                        outs=[rs_g[g][:].opt()],
                    )

            mid.close()  # free xgT/act before the tail

            # per-token symmetric int8 quantization for the return trip.
            # Per-group abs/max runs as soon as that group's ReduceScatter
            # lands, so group 0's reduction hides under group 1's collective.
            with contextlib.ExitStack() as ph:
                op_ = ph.enter_context(tc.tile_pool(name="outp", bufs=2))
                for st in range(TS // P):
                    ofbs, mxs = [], []
                    for g in range(NG):
                        ofb = op_.tile([P, HG], F16, tag=f"ofb{g}",
                                       name=f"ofb{g}_{st}")
                        nc.sync.dma_start(
                            ofb, rs_g[g][st * P : (st + 1) * P, :]
                        )
                        ab = op_.tile([P, HG], F32, tag=f"ab{g}",
                                      name=f"ab{g}_{st}")
                        nc.scalar.activation(ab, ofb, AF.Abs)
                        mxg = op_.tile([P, 1], F32, tag=f"mxg{g}",
                                       name=f"mxg{g}_{st}")
                        nc.vector.reduce_max(mxg, ab, axis=AX.X)
                        ofbs.append(ofb)
                        mxs.append(mxg)
                    mx = op_.tile([P, 1], F32, tag="mx")
                    if NG == 1:
                        nc.vector.tensor_copy(mx, mxs[0])
                    else:
                        nc.vector.tensor_tensor(mx, mxs[0], mxs[1],
                                                op=ALU.max)
                        for g in range(2, NG):
                            nc.vector.tensor_tensor(mx, mx, mxs[g],
                                                    op=ALU.max)
                    nc.vector.tensor_scalar_add(mx, mx, 1e-30)
                    inv = op_.tile([P, 1], F32, tag="inv")
                    nc.vector.reciprocal(inv, mx)
                    nc.vector.tensor_scalar(inv, inv, 127.0, None, op0=ALU.mult)
                    for g in range(NG):
                        q = op_.tile([P, HG], F32, tag=f"q{g}",
                                     name=f"q{g}_{st}")
                        nc.vector.tensor_scalar(q, ofbs[g], inv, None,
                                                op0=ALU.mult)
                        qi = op_.tile([P, HG], mybir.dt.int8, tag=f"qi{g}",
                                      name=f"qi{g}_{st}")
                        nc.vector.tensor_copy(qi, q)
                        nc.sync.dma_start(
                            out_d[st * P : (st + 1) * P,
                                  g * HG : (g + 1) * HG], qi
                        )
                    sc = op_.tile([P, 1], F32, tag="sc")
                    nc.vector.tensor_scalar(
                        sc, mx, 1.0 / 127.0, None, op0=ALU.mult
                    )
                    nc.sync.dma_start(
                        out_d[st * P : (st + 1) * P, H : H + 4].bitcast(F32), sc
                    )

    nc.compile()
    return nc


# ---------------------------------------------------------------------------
# dispatch: jit once, keep weights device-resident across calls


def _fingerprint(a: np.ndarray) -> bytes:
    h = hashlib.blake2b(digest_size=16)
    h.update(repr((a.shape, str(a.dtype))).encode())
    b = a.reshape(-1)
    step = max(1, b.size // 262144)
    h.update(np.ascontiguousarray(b[::step]).tobytes())
    return h.digest()


class _State:
    def __init__(self, C):
        install_neuronx_cc_hook()
        self.C = C
        self.nc = build_moe(C)
        nc = self.nc
        devices = jax.devices()[:NCORES]
        assert len(devices) == NCORES, f"need {NCORES} devices"
        self.mesh = Mesh(np.asarray(devices), ("core",))
        self.sharding = NamedSharding(self.mesh, PartitionSpec("core"))

        in_names, in_avals, out_names, out_avals = [], [], [], []
        pname = nc.partition_id_tensor.name if nc.partition_id_tensor else None
        for alloc in nc.m.functions[0].allocations:
            if not isinstance(alloc, mybir.MemoryLocationSet):
                continue
            name = alloc.memorylocations[0].name
            if alloc.kind == "ExternalInput":
                if name != pname:
                    in_names.append(name)
                    in_avals.append((tuple(alloc.tensor_shape),
                                     mybir.dt.np(alloc.dtype)))
            elif alloc.kind == "ExternalOutput":
                out_names.append(name)
                out_avals.append(
                    jax.core.ShapedArray(
                        tuple(alloc.tensor_shape), mybir.dt.np(alloc.dtype)
                    )
                )
        self.in_names = in_names
        bind_names = tuple(in_names) + ((pname,) if pname else ())
        out_avals = tuple(out_avals)
        out_names = tuple(out_names)

        def _body(*args):
            ops = list(args)
            if pname:
                ops.append(partition_id_tensor())
            outs = _bass_exec_p.bind(
                *ops,
                out_avals=out_avals,
                in_names=bind_names,
                out_names=out_names,
                lowering_input_output_aliases=(),
                sim_require_finite=True,
                sim_require_nnan=True,
                nc=nc,
            )
            return tuple(outs)

        n_in = len(in_names)
        sm = shard_map(
            _body,
            mesh=self.mesh,
            in_specs=(PartitionSpec("core"),) * n_in,
            out_specs=(PartitionSpec("core"),),
            check_rep=False,
        )

        def compile_fn():
            jt = jax.jit(sm, keep_unused=True)
            args = [
                jax.ShapeDtypeStruct(
                    (NCORES * shape[0],) + tuple(shape[1:]), dt,
                    sharding=self.sharding,
                )
                for shape, dt in in_avals
            ]
            return jt.lower(*args).compile()

        self.jitted = fast_dispatch_compile(compile_fn)
        self._wcache = {}  # name -> (src_ref, fingerprint, device_array)

    def _cached(self, name, src, prep):
        ent = self._wcache.get(name)
        if ent is not None and ent[0] is src:
            return ent[2]
        fp = _fingerprint(src)
        if ent is not None and ent[1] == fp:
            self._wcache[name] = (src, fp, ent[2])
            return ent[2]
        arr = jax.device_put(prep(src), self.sharding)
        self._wcache[name] = (src, fp, arr)
        return arr

    def weights(self, w_gate, w_up, w_down):
        IC, HC = I0 // P, H0 // P

        def prep_1(w):  # [E, I, H] -> concat_e [128, IC*HC*128], [p,ic,hc,i]
            w = np.asarray(w, np.float32).astype(NP16)
            parts = [
                np.ascontiguousarray(
                    w[e].reshape(IC, P, HC, P).transpose(3, 0, 2, 1)
                ).reshape(P, IC * HC * P)
                for e in range(NCORES)
            ]
            return np.concatenate(parts, axis=0)

        def prep_2(w):  # [E, H, I] -> concat_e [128, HC*IC*128], [p,hc,ic,h]
            w = np.asarray(w, np.float32).astype(NP16)
            parts = [
                np.ascontiguousarray(
                    w[e].reshape(HC, P, IC, P).transpose(3, 0, 2, 1)
                ).reshape(P, HC * IC * P)
                for e in range(NCORES)
            ]
            return np.concatenate(parts, axis=0)

        return {
            "wg": self._cached("wg", w_gate, prep_1),
            "wu": self._cached("wu", w_up, prep_1),
            "wd": self._cached("wd", w_down, prep_2),
        }


_STATE = None


def _get_state(C=576):
    global _STATE
    if _STATE is None or _STATE.C < C:
        _STATE = _State(C)
    return _STATE


def _host_router(x, w_router):
    """Exact fp32 top-2 router. Returns (token lists, combine weights) per
    expert: lists[e] sorted token ids, cw[e] the matching softmax weights."""
    logits = x @ np.asarray(w_router, np.float32).T  # [T, E]
    i1 = np.argmax(logits, axis=1)
    v1 = np.take_along_axis(logits, i1[:, None], axis=1)[:, 0]
    masked = logits.copy()
    np.put_along_axis(masked, i1[:, None], -np.inf, axis=1)
    i2 = np.argmax(masked, axis=1)
    v2 = np.take_along_axis(masked, i2[:, None], axis=1)[:, 0]
    e = np.exp(v2 - v1)
    w1 = 1.0 / (1.0 + e)
    w2 = e * w1
    T, E = logits.shape
    lists, cws = [], []
    for ei in range(E):
        t1 = np.nonzero(i1 == ei)[0]
        t2 = np.nonzero(i2 == ei)[0]
        tok = np.concatenate([t1, t2])
        w = np.concatenate([w1[t1], w2[t2]])
        order = np.argsort(tok, kind="stable")
        lists.append(tok[order].astype(np.int32))
        cws.append(w[order].astype(np.float32))
    return lists, cws


_PACK_BUF = None
_PACK_POOL = None
_DEC_POOL = None


def _pack_xs(x, w_router, C):
    """[NCORES*XROWS, H] fp16: per core, its x shard plus aux rows holding
    the gather idx (int16), scatter idx (int32) and combine-weight hi/lo."""
    global _PACK_BUF, _PACK_POOL
    from concurrent.futures import ThreadPoolExecutor

    T, H = x.shape
    NJP = (C + P - 1) // P
    SLOTS = NJP * P
    if _PACK_BUF is None or _PACK_BUF.shape != (NCORES * XROWS, H):
        _PACK_BUF = np.zeros((NCORES * XROWS, H), NP16)
    if _PACK_POOL is None:
        _PACK_POOL = ThreadPoolExecutor(1)
    a = _PACK_BUF

    def _cast():
        for c in range(NCORES):
            a[c * XROWS : c * XROWS + TS, :] = x[c * TS : (c + 1) * TS]

    fut = _PACK_POOL.submit(_cast)
    lists, cws = _host_router(x, w_router)  # overlaps _cast
    maxload = max(len(l) for l in lists)
    assert maxload <= C, f"expert load {maxload} exceeds capacity {C}"
    fut.result()
    for c in range(NCORES):
        n = len(lists[c])
        gidx = np.zeros(SLOTS, np.int16)  # pads gather row 0 (cw=0 kills)
        gidx[:n] = lists[c].astype(np.int16)
        sidx = np.full(SLOTS, PAD_IDX, np.int32)  # pads skipped by bounds
        sidx[:n] = lists[c]
        cw = np.zeros(SLOTS, np.float32)
        cw[:n] = cws[c]
        r = c * XROWS + TS
        # gather idx: device reads [16, S//16] with idx[i] at [i%16, i//16]
        a[r, :SLOTS] = gidx.reshape(SLOTS // 16, 16).T.reshape(-1).view(NP16)
        a[r, SLOTS:] = 0
        # scatter idx + cw: device reads [128, NJP] with elem p*NJP+j <-
        # slot[j*128+p]
        sperm = sidx.reshape(NJP, P).T.reshape(-1)
        cwp = cw.reshape(NJP, P).T.reshape(-1)
        hi = cwp.astype(NP16)
        lo = (cwp - hi.astype(np.float32)).astype(NP16)
        a[r + 1, : 2 * SLOTS] = sperm.view(NP16)
        a[r + 1, 2 * SLOTS :] = 0
        a[r + 2, :SLOTS] = hi
        a[r + 2, SLOTS:] = 0
        a[r + 3, :SLOTS] = lo
        a[r + 3, SLOTS:] = 0
    return a, maxload


def kernel(x, w_router, w_gate, w_up, w_down, top_k):
    try:
        return _kernel_impl(x, w_router, w_gate, w_up, w_down, top_k)
    except AssertionError:
        raise
    except Exception:
        # transient device failures (e.g. NRT_EXEC_UNIT_UNRECOVERABLE) have
        # been observed on this fabric; rebuild the backend + state and
        # retry once. Any failure inside the recovery path re-raises.
        global _STATE
        _STATE = None
        try:
            import jax.extend.backend as _jeb

            _jeb.clear_backends()
        except Exception:
            pass
        try:
            jax.clear_caches()
        except Exception:
            pass
        return _kernel_impl(x, w_router, w_gate, w_up, w_down, top_k)


def _kernel_impl(x, w_router, w_gate, w_up, w_down, top_k):
    import time as _time

    t0 = _time.time()
    assert int(top_k) == 2, f"kernel specialized for top_k=2, got {top_k}"
    x = np.ascontiguousarray(np.asarray(x, dtype=np.float32))
    w_router = np.asarray(w_router)
    w_gate, w_up, w_down = (np.asarray(a) for a in (w_gate, w_up, w_down))
    T, H = x.shape
    E, I = w_gate.shape[0], w_gate.shape[1]
    assert (T, H, I, E) == (T0, H0, I0, E0), "kernel hardcoded for spec shapes"

    st = _get_state()
    try:
        packed, maxload = _pack_xs(x, w_router, st.C)
    except AssertionError:
        # an input whose max expert load exceeds capacity: rebuild bigger
        lists, _cw = _host_router(x, np.asarray(w_router, np.float32))
        ml = max(len(l) for l in lists)
        st = _get_state(((ml + 15) // 16) * 16)
        packed, maxload = _pack_xs(x, w_router, st.C)
    ws = st.weights(w_gate, w_up, w_down)
    xg = jax.device_put(packed, st.sharding)
    args = {"xs": xg, **ws}
    (out,) = st.jitted(*[args[n] for n in st.in_names])
    buf = np.asarray(out)  # int8 [T, H+4]
    scale = buf[:, H : H + 4].copy().view(np.float32)  # [T, 1]
    global _DEC_POOL
    if _DEC_POOL is None:
        from concurrent.futures import ThreadPoolExecutor

        _DEC_POOL = ThreadPoolExecutor(NCORES)
    res = np.empty((T, H), np.float32)
    rb = T // NCORES

    def _dec(b):
        r0, r1 = b * rb, (b + 1) * rb
        np.multiply(buf[r0:r1, :H], scale[r0:r1], dtype=np.float32,
                    out=res[r0:r1])

    list(_DEC_POOL.map(_dec, range(NCORES)))
    kernel._last_wall_s = _time.time() - t0
    kernel._last_exec_time_ns = None
    return res


def device_args(x, w_router, w_gate, w_up, w_down):
    """Device-resident inputs for steady-state benchmarking."""
    st = _get_state()
    packed, _ = _pack_xs(np.ascontiguousarray(np.asarray(x, np.float32)),
                         w_router, st.C)
    ws = st.weights(w_gate, w_up, w_down)
    xg = jax.device_put(packed, st.sharding)
    args = {"xs": xg, **ws}
    return st, [args[n] for n in st.in_names]
